# revision 21
# baseline (speedup 1.0000x reference)
"""Trainium2 Bass kernel for CustomPointScatter (nn_CustomPointScatter).

Reference computation:
    pillar_feat = point_features.mean(axis=1)            # [40000, 64]
    out = zeros([4, 64, 512, 512]); out[b, :, y, x] = pillar_feat

Sharding: each of the 8 cores owns one output region (b, y_half) of shape
[64, 256, 512].  The host partitions pillars by destination region, folds
the mean's 1/n_points into the gather, casts to bf16 (rel err ~6e-3,
within the 2e-2 gate), pads every group to a common multiple-of-128 size,
and hands each core its pillars plus per-pillar destination row offsets.

On device the region is laid out position-major ([256*512 (+pad), 64]) so a
pillar is one contiguous row.  The block schedule is a list of
(base_block, sup) tiles: big sup tiles stream the bulk (partition p reads
pillars [base+p*sup, base+(p+1)*sup) as one contiguous HBM stretch), and
the last blocks taper down (2,2,1,1,1,1) so the end-of-pipe drain (DVE
reduce + scatter emission + HBM write receipt) covers a small tile, not a
big one.  Per tile:
  1. one HWDGE load,
  2. five unit-count bf16 DVE halving adds (2x perf mode) reduce the
     32-point axis in place,
  3. per 128-pillar block: ACT-copy the 128 feature rows to a small tile
     (frees the load slot early) and indirect-scatter them to the bank.
Consecutive scatters rotate over independent full-size output tensors:
Tile serializes same-tensor DMA writers on full completion (~3 us each).
Destination cells are globally unique, so the banks have disjoint row
support and the host just sums them.  ExternalOutput DRAM arrives
zero-initialised (runtime contract), so only occupied rows are written.
The host reassembles the regions and transposes to [B, C, H, W].
"""

import ml_dtypes
import numpy as np

import concourse.bacc as bacc
import concourse.bass as bass
import concourse.mybir as mybir
import concourse.tile as tile
from concourse.bass_utils import run_bass_kernel_spmd

B, H, W = 4, 512, 512
N_PILLARS, N_POINTS, C = 40000, 32, 64
N_CORES = 8
P = 128
HALF = H // 2            # 256 BEV rows per core
REGION_ROWS = HALF * W   # 131072 positions per core
PAD_ROWS = P             # dump rows for padded (inactive) pillars
OUT_ROWS = REGION_ROWS + PAD_ROWS
SUP = 4                  # pillar blocks (of 128) per full super-tile
NBANKS = 4               # independent output tensors breaking scatter WAW chains
BUFS = 6
TAPER = 1                # taper the last blocks down to 1-block tiles
STAGE = 0                # stage feature rows through ACT-copied tiles
BF16 = 1                 # full-bf16 pipeline (host casts; rel err ~6e-3)
TBUFS = 5                # bufs for the small taper tiles


def make_schedule(T, sup=SUP, taper=TAPER):
    """[(base_block, tile_sup), ...] covering blocks 0..T-1.

    Small tiles go FIRST (fast ramp: the first scatter is ready after a
    ~0.5MB load + one short DVE chain instead of a full super-tile) and a
    short taper goes LAST (small end-of-pipe drain)."""
    head = [1] * 10 if taper else []
    tail = [2, 1, 1] if taper else []
    extra_blocks = sum(head) + sum(tail)
    while taper and (T - extra_blocks) % sup != 0:
        tail.append(1)
        extra_blocks += 1
    if not taper:
        assert T % sup == 0
    sched = []
    base = 0
    for s in head:
        sched.append((base, s))
        base += s
    for _ in range((T - extra_blocks) // sup):
        sched.append((base, sup))
        base += sup
    for s in tail:
        sched.append((base, s))
        base += s
    assert base == T
    return sched


def build_nc(nmax, n_points=N_POINTS, c=C, out_rows=OUT_ROWS, sup=SUP,
             bufs=BUFS, nbanks=NBANKS, taper=TAPER, stage=STAGE, bf16=BF16):
    T = nmax // P          # pillar blocks
    D = n_points * c       # full row: 2048 values
    sched = make_schedule(T, sup, taper)
    dt = mybir.dt.bfloat16 if bf16 else mybir.dt.float32
    nc = bacc.Bacc("TRN2", target_bir_lowering=False)
    pf = nc.dram_tensor("pf", [nmax, D], dt, kind="ExternalInput")
    offs = nc.dram_tensor("offs", [P, T], mybir.dt.int32, kind="ExternalInput")
    banks = [
        nc.dram_tensor(f"out{k}", [out_rows, c], dt, kind="ExternalOutput")
        for k in range(nbanks)
    ]
    with tile.TileContext(nc) as tc:
        with (
            tc.tile_pool(name="io", bufs=bufs) as io_pool,
            tc.tile_pool(name="misc", bufs=1) as misc,
        ):
            offs_sb = misc.tile([P, T], mybir.dt.int32)
            nc.sync.dma_start(out=offs_sb[:], in_=offs[:])
            for base, s in sched:
                rows = slice(base * P, (base + s) * P)
                sb = io_pool.tile([P, s * D], dt, tag=f"sb{s}",
                                  bufs=bufs if s == sup else TBUFS)
                v = sb[:].rearrange("p (blk w) -> p blk w", w=D)
                # pillar j = base*128 + p*s + blk -> partition p, block blk
                # (each partition reads one contiguous HBM stretch)
                nc.sync.dma_start(
                    out=v,
                    in_=pf[rows, :].rearrange("(p blk) w -> p blk w", blk=s),
                )
                # 1/n_points is folded into the host-side gather.
                w = D
                while w > c:
                    w //= 2
                    nc.vector.tensor_add(
                        out=v[:, :, :w], in0=v[:, :, :w], in1=v[:, :, w:2 * w]
                    )
                for blk in range(s):
                    g = base + blk
                    if stage:
                        feat = io_pool.tile([P, c], dt, tag="feat", bufs=12)
                        nc.scalar.copy(out=feat[:], in_=v[:, blk, :c])
                        src = feat[:]
                    else:
                        src = v[:, blk, :c]
                    nc.gpsimd.indirect_dma_start(
                        out=banks[g % nbanks][:],
                        out_offset=bass.IndirectOffsetOnAxis(
                            ap=offs_sb[:, g:g + 1], axis=0
                        ),
                        in_=src,
                        in_offset=None,
                    )
    nc.finalize()  # Bacc.compile(): splits multi-waits for TRN2 codegen
    return nc


def shard_inputs(point_features, voxel_coords, sup=SUP, taper=TAPER,
                 bf16=BF16):
    pf = np.ascontiguousarray(
        np.asarray(point_features, dtype=np.float32).reshape(N_PILLARS, N_POINTS * C)
    )
    np_dt = ml_dtypes.bfloat16 if bf16 else np.float32
    vc = np.asarray(voxel_coords)
    b = vc[:, 0].astype(np.int64)
    y = vc[:, 2].astype(np.int64)
    x = vc[:, 3].astype(np.int64)
    upper = (y >= HALF).astype(np.int64)
    region = b * 2 + upper
    off = (y - upper * HALF) * W + x  # row offset within the owned region
    idx_r = [np.nonzero(region == r)[0] for r in range(N_CORES)]
    nmax = max(len(ix) for ix in idx_r)
    nmax = max(P, ((nmax + P - 1) // P) * P)
    if not taper:
        a2 = sup * P
        nmax = ((nmax + a2 - 1) // a2) * a2
    T = nmax // P
    sched = make_schedule(T, sup, taper)
    inv_np = np.float32(1.0 / N_POINTS)
    in_maps = []
    for r in range(N_CORES):
        ix = idx_r[r]
        pf_r = np.zeros((nmax, N_POINTS * C), np_dt)
        # fold the mean's 1/n_points into the gather
        pf_r[: len(ix)] = (pf[ix] * inv_np).astype(np_dt)
        offs_r = np.full(nmax, REGION_ROWS, np.int32)  # pad rows -> dump row
        offs_r[: len(ix)] = off[ix].astype(np.int32)
        # tile (base, s): pillar j = base*128 + p*s + blk -> offs_arr[p, base+blk]
        offs_arr = np.empty((P, T), np.int32)
        for base, s in sched:
            seg = offs_r[base * P:(base + s) * P].reshape(P, s)
            offs_arr[:, base:base + s] = seg
        in_maps.append({"pf": pf_r, "offs": np.ascontiguousarray(offs_arr)})
    return in_maps, nmax


def assemble(results):
    out = np.empty((B, C, H, W), np.float32)
    for r in range(N_CORES):
        names = sorted(results[r])       # out0..out{nbanks-1}
        region = np.asarray(results[r][names[0]], dtype=np.float32)
        for name in names[1:]:
            # banks: disjoint row support
            region = region + np.asarray(results[r][name], dtype=np.float32)
        o = region[:REGION_ROWS].reshape(HALF, W, C)
        b_, half = divmod(r, 2)
        out[b_, :, half * HALF:(half + 1) * HALF, :] = o.transpose(2, 0, 1)
    return out


def run(point_features, voxel_coords, trace=False, sup=SUP, bufs=BUFS,
        nbanks=NBANKS, taper=TAPER, stage=STAGE, bf16=BF16, **spmd_kwargs):
    in_maps, nmax = shard_inputs(point_features, voxel_coords,
                                 sup=sup, taper=taper, bf16=bf16)
    nc = build_nc(nmax, sup=sup, bufs=bufs, nbanks=nbanks, taper=taper,
                  stage=stage, bf16=bf16)
    br = run_bass_kernel_spmd(
        nc, in_maps, list(range(N_CORES)), trace=trace, **spmd_kwargs
    )
    return assemble(br.results), br


def kernel(point_features, voxel_coords):
    out, _ = run(point_features, voxel_coords)
    return out
